# revision 37
# baseline (speedup 1.0000x reference)
"""Trainium2 Bass kernel for nn_AttentionModel (4-layer transformer encoder).

Sharding: 8 cores = 4 batches x 2 sequence halves (1024 tokens/core).
Per layer each core AllGathers the feature-major hidden state within its
batch pair to get full-sequence k/v; attention + FFN are otherwise local.
"""
import sys

sys.path.insert(0, "/opt/trn_rl_repo")

import numpy as np
import ml_dtypes

import concourse.bass as bass
import concourse.tile as tile
from concourse import mybir, bacc
from concourse.masks import make_identity

DT = mybir.dt
AF = mybir.ActivationFunctionType
OP = mybir.AluOpType

P = 128
L, D, H, DFF, DIN, DOUT = 4, 512, 8, 2048, 64, 10
DK = D // H          # 64
NP = H // 2          # head pairs = 4
DS = D // P          # d slabs = 4
FS = DFF // P        # dff slabs = 16
TOK = 1024           # own tokens per core
NT = TOK // P        # 8 own token tiles
NG = 2               # token groups (512 each) per core
EPS = 1e-5
RSQRT_MAGIC = 0x5F3759DF

# Schraudolph fast-exp on DVE for a subset of softmax tiles: ACT's exp
# stream is the attention-phase bottleneck (1 elem/cycle/lane) while DVE
# has slack there. i = x*(0.125*log2e*2^23) + (127*2^23 - 486408);
# bitcast(i) ~ exp(0.125*x) with ~3% sawtooth rel err that averages out
# in the softmax weighted sum. Scores are |x| < ~100 so the exponent
# field can't under/overflow.
FEXP_C1 = 0.125 * 1.4426950408889634 * 8388608.0
FEXP_C2 = float(127 * 8388608 - 486408)
# (q, tt) softmax tiles routed to DVE. Measured on HW (iters-slope A/B):
# offloading 32 tiles/pass costs +7% (1.271 -> 1.364 ms/pass) — attention
# is not ACT-bound enough, and the DVE-queue insertion delays the
# normalize/LN ops PE waits on. TimelineSim agreed. Kept empty; the code
# path remains for future re-tuning (accuracy was fine: rel err 2.65e-3).
FEXP_QTT = frozenset()


def _bcast_ap(ap_1d, parts=P):
    """Broadcast a 1-D DRAM AP across partitions (DMA-side replication)."""
    return bass.AP(
        tensor=ap_1d.tensor, offset=ap_1d.offset, ap=[[0, parts], *ap_1d.ap]
    )


def _rsqrt_newton(nc, pool, var_ap, out_ap, w=1):
    """out = 1/sqrt(var + EPS) on DVE only (no ACT table switch). [P,w] f32."""
    x = pool.tile([P, w], DT.float32, tag=f"rs_x{w}")
    nc.vector.tensor_scalar_add(x[:], var_ap, EPS)
    t_i = pool.tile([P, w], DT.int32, tag=f"rs_i{w}")
    nc.vector.tensor_scalar(
        out=t_i[:], in0=x[:].bitcast(DT.int32), scalar1=1, scalar2=None,
        op0=OP.logical_shift_right,
    )
    nc.vector.tensor_scalar(
        out=out_ap.bitcast(DT.int32), in0=t_i[:], scalar1=-1, scalar2=RSQRT_MAGIC,
        op0=OP.mult, op1=OP.add,
    )
    tmp = pool.tile([P, w], DT.float32, tag=f"rs_t{w}")
    for _ in range(2):
        nc.vector.tensor_mul(tmp[:], out_ap, out_ap)
        nc.vector.tensor_mul(tmp[:], tmp[:], x[:])
        nc.vector.tensor_scalar(
            out=tmp[:], in0=tmp[:], scalar1=-0.5, scalar2=1.5, op0=OP.mult, op1=OP.add
        )
        nc.vector.tensor_mul(out_ap, out_ap, tmp[:])


def _ln_stats(nc, st, xs, tagsfx=""):
    """bn_stats/aggr for a group of [P, D] tiles -> packed [P, n, 2] mean/var."""
    n = len(xs)
    mvg = st.tile([P, n, 2], DT.float32, tag=f"ln_mvg{tagsfx}", name=f"mvg{tagsfx}")
    for i, x_ap in enumerate(xs):
        stats = st.tile([P, 6], DT.float32, tag="ln_st")
        nc.vector.bn_stats(stats[:], x_ap)
        nc.vector.bn_aggr(mvg[:, i, :], stats[:])
    return mvg


def _ln_finish(nc, st, mvg, xs, g_bc, b_bc, outs, tagsfx=""):
    """Vectorized Newton rsqrt + normalize/affine for the group."""
    n = len(xs)
    rsg = st.tile([P, n], DT.float32, tag=f"ln_rsg{tagsfx}", name=f"rsg{tagsfx}")
    _rsqrt_newton(nc, st, mvg[:, :, 1], rsg[:], w=n)
    for i, (x_ap, out_ap) in enumerate(zip(xs, outs)):
        nc.vector.tensor_scalar(
            out=x_ap, in0=x_ap, scalar1=mvg[:, i, 0:1], scalar2=rsg[:, i : i + 1],
            op0=OP.subtract, op1=OP.mult,
        )
        nc.vector.tensor_mul(x_ap, x_ap, g_bc)
        nc.vector.tensor_add(out_ap, x_ap, b_bc)


def _layernorm_group(nc, st, xs, g_bc, b_bc, outs, tagsfx=""):
    mvg = _ln_stats(nc, st, xs, tagsfx)
    _ln_finish(nc, st, mvg, xs, g_bc, b_bc, outs, tagsfx)


def _layernorm(nc, st, x_ap, g_bc, b_bc, out_ap):
    """LN over free dim D; x_ap [P, D] f32 -> out_ap (f32)."""
    stats = st.tile([P, 6], DT.float32, tag="ln_st")
    nc.vector.bn_stats(stats[:], x_ap)
    mv = st.tile([P, 2], DT.float32, tag="ln_mv")
    nc.vector.bn_aggr(mv[:], stats[:])
    rstd = st.tile([P, 1], DT.float32, tag="ln_rs")
    _rsqrt_newton(nc, st, mv[:, 1:2], rstd[:])
    nc.vector.tensor_scalar(
        out=x_ap, in0=x_ap, scalar1=mv[:, 0:1], scalar2=rstd[:],
        op0=OP.subtract, op1=OP.mult,
    )
    nc.vector.tensor_mul(x_ap, x_ap, g_bc)
    nc.vector.tensor_add(out_ap, x_ap, b_bc)


def build(group_size=2, fake_cc=False, iters=1):
    """Build the SPMD program. group_size=2 -> pairwise AG (real); 1 -> no-op
    AG (single-core numerics testing, S=1024). fake_cc replaces collectives
    with local DMA copies (perf-model variant for TimelineSim). iters>1
    repeats the full forward pass back-to-back inside the NEFF (identical
    output; used to measure steady-state per-pass device time by slope)."""
    GS = group_size
    S = GS * TOK                 # attended sequence length
    NC_TK = S // P               # tk tiles (16 or 8)
    NCH = S // 512               # 512-token key chunks (4 or 2)
    if GS == 2:
        groups = [[0, 1], [2, 3], [4, 5], [6, 7]]
    else:
        groups = [[c] for c in range(8)]

    nc = bacc.Bacc("TRN2", target_bir_lowering=False, debug=False)

    # ---- DRAM I/O ----
    xT_d = nc.dram_tensor("xT", [DIN, TOK], DT.bfloat16, kind="ExternalInput")
    Wp_d = nc.dram_tensor("Wp", [DIN, D], DT.bfloat16, kind="ExternalInput")
    bp_d = nc.dram_tensor("bp", [D], DT.float32, kind="ExternalInput")
    Wq_d = nc.dram_tensor("Wq", [L, D, D], DT.bfloat16, kind="ExternalInput")
    Wk_d = nc.dram_tensor("Wk", [L, D, D], DT.bfloat16, kind="ExternalInput")
    Wv_d = nc.dram_tensor("Wv", [L, D, D], DT.bfloat16, kind="ExternalInput")
    Wo_d = nc.dram_tensor("Wo", [L, D, D], DT.bfloat16, kind="ExternalInput")
    bq_d = nc.dram_tensor("bq", [L, D], DT.float32, kind="ExternalInput")
    bk_d = nc.dram_tensor("bk", [L, D], DT.float32, kind="ExternalInput")
    bv_d = nc.dram_tensor("bv", [L, D], DT.float32, kind="ExternalInput")
    bo_d = nc.dram_tensor("bo", [L, D], DT.float32, kind="ExternalInput")
    lg_d = nc.dram_tensor("ln_g", [L, D], DT.float32, kind="ExternalInput")
    lb_d = nc.dram_tensor("ln_b", [L, D], DT.float32, kind="ExternalInput")
    W1_d = nc.dram_tensor("W1", [L, D, DFF], DT.bfloat16, kind="ExternalInput")
    b1_d = nc.dram_tensor("b1", [L, DFF], DT.float32, kind="ExternalInput")
    W2_d = nc.dram_tensor("W2", [L, DFF, D], DT.bfloat16, kind="ExternalInput")
    b2_d = nc.dram_tensor("b2", [L, D], DT.float32, kind="ExternalInput")
    out_d = nc.dram_tensor("pooled", [D], DT.float32, kind="ExternalOutput")

    # AG bounce buffers (per layer)
    ag_in = [
        nc.dram_tensor(f"ag_in{l}", [NG, DS, P, 512], DT.bfloat16) for l in range(L)
    ]
    ag_out = [
        nc.dram_tensor(f"ag_out{l}", [NG, GS, DS, P, 512], DT.bfloat16)
        for l in range(L)
    ]

    from contextlib import ExitStack

    with tile.TileContext(nc) as tc:
        with ExitStack() as _es:
            sing = _es.enter_context(tc.tile_pool(name="sing", bufs=1))
            hp = _es.enter_context(tc.tile_pool(name="hp", bufs=2))
            hTp = _es.enter_context(tc.tile_pool(name="hTp", bufs=1))
            kvp = _es.enter_context(tc.tile_pool(name="kvp", bufs=1))
            chk = _es.enter_context(tc.tile_pool(name="chk", bufs=3))
            wts = _es.enter_context(tc.tile_pool(name="wts", bufs=1))
            biasp = _es.enter_context(tc.tile_pool(name="bias", bufs=1))
            lnp = _es.enter_context(tc.tile_pool(name="lnp", bufs=2))
            expp = _es.enter_context(tc.tile_pool(name="expp", bufs=4))
            rtp = _es.enter_context(tc.tile_pool(name="rtp", bufs=4))
            rcp = _es.enter_context(tc.tile_pool(name="rcp", bufs=2))
            st = _es.enter_context(tc.tile_pool(name="st", bufs=4))
            xw = _es.enter_context(tc.tile_pool(name="xw", bufs=4))
            # ---- singles ----
            ident = sing.tile([P, P], DT.float32)
            make_identity(nc, ident[:])
            ones_col = sing.tile([P, 1], DT.float32)
            nc.vector.memset(ones_col[:], 1.0)
            bp_bc = sing.tile([P, D], DT.float32)
            nc.sync.dma_start(bp_bc[:], _bcast_ap(bp_d[:]))
            xT_sb = sing.tile([DIN, TOK], DT.bfloat16)
            nc.sync.dma_start(xT_sb[:], xT_d[:])
            Wp_sb = sing.tile([DIN, D], DT.bfloat16)
            nc.sync.dma_start(Wp_sb[:], Wp_d[:])
            warm = sing.tile([1, 2], DT.float32)
            nc.vector.memset(warm[:], 0.0)
            nc.scalar.activation(warm[:, 1:2], warm[:, 0:1], AF.Exp)
            nc.vector.tensor_copy(warm[:, 0:1], warm[:, 1:2])

            # persistent activation tiles
            hT_sb = hTp.tile([P, DS, TOK], DT.bfloat16, tag="hT")
            hpT_sb = hTp.tile([P, DS, TOK], DT.bfloat16, tag="hpT")
            qT_sb = kvp.tile([P, NP, TOK], DT.bfloat16, tag="qT")
            kT_sb = kvp.tile([P, NP, S], DT.bfloat16, tag="kT")
            v_sb = kvp.tile([P, NC_TK, H, DK + 1], DT.bfloat16, tag="v")
            ctxT_sb = kvp.tile([P, NP, TOK], DT.bfloat16, tag="ctxT")

            def transpose_to(dst_sb, src_ap, tt, ps_pool, tag="ps_t"):
                """PE-transpose a [P, D] f32 tile into dst_sb[:, :, tt*128...]"""
                ps_t = ps_pool.tile([P, D], DT.float32, tag=tag,
                                    name=f"tr_{dst_sb.tensor.name}_{tt}")
                for s in range(DS):
                    nc.tensor.transpose(
                        ps_t[:, s * P : (s + 1) * P],
                        src_ap[:, s * P : (s + 1) * P],
                        ident[:],
                    )
                nc.vector.tensor_copy(
                    dst_sb[:, :, tt * P : (tt + 1) * P].rearrange("p s t -> p s t"),
                    ps_t[:].rearrange("p (s t) -> p s t", s=DS),
                )

            def start_ag(l, g):
                nc.sync.dma_start(
                    ag_in[l][g].rearrange("s p t -> p s t"),
                    hT_sb[:, :, g * 512 : (g + 1) * 512],
                )
                if fake_cc:
                    for r in range(GS):
                        nc.sync.dma_start(ag_out[l][g][r], ag_in[l][g])
                else:
                    nc.gpsimd.collective_compute(
                        "AllGather",
                        OP.bypass,
                        replica_groups=groups,
                        ins=[ag_in[l][g].opt()],
                        outs=[ag_out[l][g].opt()],
                    )

            for _it in range(iters):
              # ---- input projection -> h(0), hT, AG(0) ----
              h_cur = hp.tile([P, NT, D], DT.float32, tag="h", name=f"h_cur{_it}")
              with tc.tile_pool(name=f"ps_x{_it}", bufs=2, space="PSUM") as ps_x:
                for g in range(NG):
                    for j in range(4):
                        tt = 4 * g + j
                        ps = ps_x.tile([P, D], DT.float32, tag="ps_h0")
                        nc.tensor.matmul(
                            ps[:], xT_sb[:, tt * P : (tt + 1) * P], Wp_sb[:],
                            start=True, stop=True,
                        )
                        nc.vector.tensor_add(h_cur[:, tt, :], ps[:], bp_bc[:])
                        transpose_to(hT_sb, h_cur[:, tt, :], tt, ps_x)
                    start_ag(0, g)

              # ---- layers ----
              for l in range(L):
                # -- weights / params for this layer --
                Wq_sb = wts.tile([P, DS, D], DT.bfloat16, tag="Wq")
                Wk_sb = wts.tile([P, DS, D], DT.bfloat16, tag="Wk")
                Wv_sb = wts.tile([P, DS, D], DT.bfloat16, tag="Wv")
                Wo_sb = wts.tile([P, DS, D], DT.bfloat16, tag="Wo")
                for w_sb, w_d in ((Wq_sb, Wq_d), (Wk_sb, Wk_d), (Wv_sb, Wv_d),
                                  (Wo_sb, Wo_d)):
                    nc.sync.dma_start(
                        w_sb[:], w_d[l].rearrange("(s p) e -> p s e", p=P)
                    )
                bq_c = biasp.tile([P, DS], DT.float32, tag="bq")
                nc.sync.dma_start(bq_c[:], bq_d[l].rearrange("(s p) -> p s", p=P))
                bk_c = biasp.tile([P, DS], DT.float32, tag="bk")
                nc.sync.dma_start(bk_c[:], bk_d[l].rearrange("(s p) -> p s", p=P))
                bv_bc = biasp.tile([P, D], DT.float32, tag="bv")
                nc.sync.dma_start(bv_bc[:], _bcast_ap(bv_d[l]))

                nc.gpsimd.memset(v_sb[:, :, :, DK : DK + 1], 1.0)

                # -- qT (own tokens) --
                with tc.tile_pool(name=f"ps_q{_it}_{l}", bufs=2, space="PSUM") as ps_q:
                    for p_ in range(NP):
                        for q in range(2):
                            ps = ps_q.tile([P, 512], DT.float32, tag="ps_qT")
                            for s in range(DS):
                                nc.tensor.matmul(
                                    ps[:],
                                    Wq_sb[:, s, p_ * P : (p_ + 1) * P],
                                    hT_sb[:, s, q * 512 : (q + 1) * 512],
                                    start=(s == 0), stop=(s == DS - 1),
                                )
                            nc.vector.tensor_scalar_add(
                                qT_sb[:, p_, q * 512 : (q + 1) * 512],
                                ps[:], bq_c[:, p_ : p_ + 1],
                            )

                    # -- k/v from gathered full-sequence hT --
                    # key-column order = arrival order: group-0 chunks (whose
                    # AllGather completes earliest) take the low kT columns, so
                    # attention's ctx accumulation (tt ascending) never stalls
                    # on the late group-1 gather. Key permutation is safe:
                    # softmax is order-invariant and v uses the same order.
                    chunks = sorted(
                        [(ch, g) for ch in range(GS) for g in range(NG)],
                        key=lambda t: (t[1], t[0]),
                    )
                    for c, (ch, g) in enumerate(chunks):
                        hTf = chk.tile([P, DS, 512], DT.bfloat16, tag="hTf")
                        nc.sync.dma_start(
                            hTf[:], ag_out[l][g][ch].rearrange("s p t -> p s t")
                        )
                        for p_ in range(NP):
                            ps = ps_q.tile([P, 512], DT.float32, tag="ps_qT")
                            for s in range(DS):
                                nc.tensor.matmul(
                                    ps[:],
                                    Wk_sb[:, s, p_ * P : (p_ + 1) * P],
                                    hTf[:, s, :],
                                    start=(s == 0), stop=(s == DS - 1),
                                )
                            nc.vector.tensor_scalar_add(
                                kT_sb[:, p_, c * 512 : (c + 1) * 512],
                                ps[:], bk_c[:, p_ : p_ + 1],
                            )
                        for j in range(4):
                            tt = 4 * c + j
                            ps = ps_q.tile([P, 512], DT.float32, tag="ps_qT")
                            for s in range(DS):
                                nc.tensor.matmul(
                                    ps[:],
                                    hTf[:, s, j * P : (j + 1) * P],
                                    Wv_sb[:, s, :],
                                    start=(s == 0), stop=(s == DS - 1),
                                )
                            nc.vector.tensor_add(
                                v_sb[:, tt, :, 0:DK].rearrange("p h d -> p h d"),
                                ps[:].rearrange("p (h d) -> p h d", h=H),
                                bv_bc[:].rearrange("p (h d) -> p h d", h=H),
                            )

                # -- deferred big-weight loads: issued after the latency-
                # critical AG/kv-chunk DMAs so they don't contend for queues;
                # they complete during the long attention phase --
                W1_sb = wts.tile([P, DS, DFF], DT.bfloat16, tag="W1")
                nc.sync.dma_start(W1_sb[:], W1_d[l].rearrange("(s p) e -> p s e", p=P))
                W2_sb = wts.tile([P, FS, D], DT.bfloat16, tag="W2")
                nc.sync.dma_start(W2_sb[:], W2_d[l].rearrange("(s p) e -> p s e", p=P))
                b1_c = biasp.tile([P, FS], DT.float32, tag="b1")
                nc.sync.dma_start(b1_c[:], b1_d[l].rearrange("(s p) -> p s", p=P))
                bo_bc = biasp.tile([P, D], DT.float32, tag="bo")
                nc.sync.dma_start(bo_bc[:], _bcast_ap(bo_d[l]))
                b2_bc = biasp.tile([P, D], DT.float32, tag="b2")
                nc.sync.dma_start(b2_bc[:], _bcast_ap(b2_d[l]))
                lg_bc = lnp.tile([P, D], DT.float32, tag="lg")
                nc.sync.dma_start(lg_bc[:], _bcast_ap(lg_d[l]))
                lb_bc = lnp.tile([P, D], DT.float32, tag="lb")
                nc.sync.dma_start(lb_bc[:], _bcast_ap(lb_d[l]))

                # -- attention (q-outer; head pairs row-packed on PE) --
                # attn_out/LN1/h'-transposes for token group q are emitted
                # right after q's heads so they overlap the other q's
                # ACT-bound softmax work.
                h_ln1 = hp.tile([P, NT, D], DT.float32, tag="h")
                xs_g0, mvg_g0 = [], []
                with (
                    tc.tile_pool(name=f"ps_s{_it}_{l}", bufs=2, space="PSUM") as ps_s,
                    tc.tile_pool(name=f"ps_cx{_it}_{l}", bufs=2, space="PSUM") as ps_cx,
                ):
                    for q in range(2):
                        for p_ in range(NP):
                            if q == 1 and p_ == 1:
                                # group-0 attn_out + residual + stats ride the
                                # fast-release cx slots, overlapping q1's
                                # ACT-bound softmax (transposes/Newton stay in
                                # the FFN block: they are LN-gated)
                                for j4 in range(4):
                                    aops = ps_cx.tile(
                                        [P, D], DT.float32,
                                        tag=("ps_cx0", "ps_cx1")[j4 % 2],
                                        name=f"ao_{l}_{j4}",
                                    )
                                    for p2 in range(NP):
                                        nc.tensor.matmul(
                                            aops[:],
                                            ctxT_sb[:, p2, j4 * P : (j4 + 1) * P],
                                            Wo_sb[:, p2, :],
                                            start=(p2 == 0), stop=(p2 == NP - 1),
                                        )
                                    x = xw.tile([P, D], DT.float32, tag="x1",
                                                name=f"x_{l}_{j4}")
                                    nc.vector.tensor_add(
                                        x[:], aops[:], h_cur[:, j4, :]
                                    )
                                    nc.vector.tensor_add(x[:], x[:], bo_bc[:])
                                    xs_g0.append(x)
                                mvg_g0.append(
                                    _ln_stats(nc, st,
                                              [x[:] for x in xs_g0], tagsfx="a")
                                )
                                _ln_finish(
                                    nc, st, mvg_g0[0], [x[:] for x in xs_g0],
                                    lg_bc[:], lb_bc[:],
                                    [h_ln1[:, j4, :] for j4 in range(4)],
                                    tagsfx="a",
                                )
                            if q == 1 and p_ == NP - 1:
                                # group-0 h' transposes: inputs ready (LN-g0
                                # finished mid-q1), fast release; priority just
                                # below pairs 0-2 so the exp stream never
                                # starves. Frees FFN1(g0) to start at FFN-open.
                                for j4 in range(4):
                                    trp = ps_cx.tile(
                                        [P, D], DT.float32,
                                        tag=("ps_cx0", "ps_cx1")[j4 % 2],
                                        name=f"trg0_{l}_{j4}",
                                    )
                                    for s in range(DS):
                                        nc.tensor.transpose(
                                            trp[:, s * P : (s + 1) * P],
                                            h_ln1[:, j4, s * P : (s + 1) * P],
                                            ident[:],
                                        )
                                    nc.vector.tensor_copy(
                                        hpT_sb[:, :, j4 * P : (j4 + 1) * P],
                                        trp[:].rearrange("p (s t) -> p s t", s=DS),
                                    )
                            ctx0 = ps_cx.tile([P, 512], DT.float32, tag="ps_cx0",
                                              name=f"cx0_{l}_{q}_{p_}")
                            ctx1 = ps_cx.tile([P, 512], DT.float32, tag="ps_cx1",
                                              name=f"cx1_{l}_{q}_{p_}")
                            ctxs = (ctx0, ctx1)
                            for tt in range(NC_TK):
                                sc_ps = ps_s.tile([P, 1024], DT.float32, tag="ps_sc",
                                                  name=f"sc_{l}_{q}_{p_}_{tt}")
                                for sub in range(2):
                                    nc.tensor.matmul(
                                        sc_ps[:, sub * 512 : (sub + 1) * 512],
                                        kT_sb[
                                            sub * DK : (sub + 1) * DK,
                                            p_, tt * P : (tt + 1) * P,
                                        ],
                                        qT_sb[
                                            sub * DK : (sub + 1) * DK,
                                            p_, q * 512 : (q + 1) * 512,
                                        ],
                                        start=True, stop=True,
                                        tile_position=(sub * DK, 0),
                                    )
                                eT = expp.tile([P, 1024], DT.bfloat16, tag="expT")
                                if (q, tt) in FEXP_QTT:
                                    # bufs=1: producer+consumer both on DVE
                                    # in-order, so no WAR stall
                                    ei = expp.tile([P, 1024], DT.int32,
                                                   tag="expI", bufs=1)
                                    nc.vector.tensor_scalar(
                                        out=ei[:], in0=sc_ps[:],
                                        scalar1=FEXP_C1, scalar2=FEXP_C2,
                                        op0=OP.mult, op1=OP.add,
                                    )
                                    nc.vector.tensor_copy(
                                        eT[:], ei[:].bitcast(DT.float32)
                                    )
                                else:
                                    nc.scalar.activation(eT[:], sc_ps[:],
                                                         AF.Exp, scale=0.125)
                                for sub in range(2):
                                    nc.tensor.matmul(
                                        ctxs[sub][0 : DK + 1, :],
                                        v_sb[:, tt, 2 * p_ + sub, :],
                                        eT[:, sub * 512 : (sub + 1) * 512],
                                        start=(tt == 0), stop=(tt == NC_TK - 1),
                                    )
                            for sub in range(2):
                                recip = rcp.tile([1, 512], DT.float32, tag="recip")
                                nc.vector.reciprocal(
                                    recip[:], ctxs[sub][DK : DK + 1, :]
                                )
                                rb = rcp.tile([DK, 512], DT.float32, tag="rb")
                                nc.gpsimd.partition_broadcast(rb[:], recip[:])
                                nc.vector.tensor_mul(
                                    ctxT_sb[
                                        sub * DK : (sub + 1) * DK,
                                        p_, q * 512 : (q + 1) * 512,
                                    ],
                                    ctxs[sub][0:DK, :], rb[:],
                                )

                # -- FFN + LN2 (+ next layer's hT/AG) --
                h_nxt = hp.tile([P, NT, D], DT.float32, tag="h")
                with (
                    tc.tile_pool(name=f"ps_d{_it}_{l}", bufs=2, space="PSUM") as ps_d,
                    tc.tile_pool(name=f"ps_r{_it}_{l}", bufs=2, space="PSUM") as ps_r,
                    tc.tile_pool(name=f"ps_f{_it}_{l}", bufs=1, space="PSUM") as ps_f,
                ):
                    for g4 in range(2):
                        if g4 == 0:
                            pass  # LN finished inside the attention window
                        else:
                            xs = []
                            for j4 in range(4):
                                tt = 4 * g4 + j4
                                ps = ps_d.tile([P, D], DT.float32, tag="ps_aot",
                                               name=f"ao_{l}_{tt}")
                                for p_ in range(NP):
                                    nc.tensor.matmul(
                                        ps[:],
                                        ctxT_sb[:, p_, tt * P : (tt + 1) * P],
                                        Wo_sb[:, p_, :],
                                        start=(p_ == 0), stop=(p_ == NP - 1),
                                    )
                                x = xw.tile([P, D], DT.float32, tag="x1",
                                            name=f"x_{l}_{tt}")
                                nc.vector.tensor_add(x[:], ps[:], h_cur[:, tt, :])
                                nc.vector.tensor_add(x[:], x[:], bo_bc[:])
                                xs.append(x)
                            _layernorm_group(
                                nc, st, [x[:] for x in xs], lg_bc[:], lb_bc[:],
                                [h_ln1[:, 4 * g4 + j4, :] for j4 in range(4)],
                                tagsfx="a",
                            )
                        if g4 == 1:
                            for j4 in range(4):
                                tt = 4 * g4 + j4
                                transpose_to(hpT_sb, h_ln1[:, tt, :], tt, ps_d,
                                             "ps_aot")
                    for g in range(NG):
                        outs = []
                        for j in range(4):
                            ot = ps_f.tile(
                                [P, D], DT.float32, tag=f"ps_ffn{j}",
                                name=f"ps_ffn{l}_{g}_{j}",
                            )
                            outs.append(ot)
                        for f in range(FS):
                            ps = ps_r.tile([P, 512], DT.float32, tag="ps_r")
                            for s in range(DS):
                                nc.tensor.matmul(
                                    ps[:],
                                    W1_sb[:, s, f * P : (f + 1) * P],
                                    hpT_sb[:, s, g * 512 : (g + 1) * 512],
                                    start=(s == 0), stop=(s == DS - 1),
                                )
                            rT = rtp.tile([P, 512], DT.bfloat16, tag="rT")
                            nc.scalar.activation(
                                rT[:], ps[:], AF.Relu, bias=b1_c[:, f : f + 1]
                            )
                            for j in range(4):
                                nc.tensor.matmul(
                                    outs[j][:],
                                    rT[:, j * P : (j + 1) * P],
                                    W2_sb[:, f, :],
                                    start=(f == 0), stop=(f == FS - 1),
                                )
                        xs2 = []
                        for j in range(4):
                            tt = 4 * g + j
                            x = xw.tile([P, D], DT.float32, tag="x1",
                                        name=f"x2_{l}_{g}_{j}")
                            nc.vector.tensor_add(x[:], outs[j][:], h_ln1[:, tt, :])
                            nc.vector.tensor_add(x[:], x[:], b2_bc[:])
                            xs2.append(x)
                        _layernorm_group(
                            nc, st, [x[:] for x in xs2], lg_bc[:], lb_bc[:],
                            [h_nxt[:, 4 * g + j, :] for j in range(4)],
                            tagsfx="f",
                        )
                        if l < L - 1:
                            for j in range(4):
                                tt = 4 * g + j
                                transpose_to(hT_sb, h_nxt[:, tt, :], tt, ps_d,
                                             "ps_aot")
                        if l < L - 1:
                            start_ag(l + 1, g)
                h_cur = h_nxt

              # ---- pooled sum over own tokens ----
              with tc.tile_pool(name=f"ps_p{_it}", bufs=1, space="PSUM") as ps_p:
                ps = ps_p.tile([1, D], DT.float32, tag="ps_pool")
                for tt in range(NT):
                    nc.tensor.matmul(
                        ps[:], ones_col[:], h_cur[:, tt, :],
                        start=(tt == 0), stop=(tt == NT - 1),
                    )
                osb = sing.tile([1, D], DT.float32, tag="osb", name=f"osb{_it}")
                nc.vector.tensor_copy(osb[:], ps[:])
                nc.sync.dma_start(out_d[None, :], osb[:])

    nc.compile()
    return nc


_BUILT = {}


def _get_built(group_size=2, iters=1):
    key = (group_size, iters)
    if key not in _BUILT:
        _BUILT[key] = build(group_size, iters=iters)
    return _BUILT[key]


class _Runner:
    """Persistent jitted SPMD executor (jit + NEFF compiled once)."""

    N = 8

    def __init__(self, group_size=2, iters=1):
        import jax
        from jax.sharding import Mesh, PartitionSpec, NamedSharding
        from jax.experimental.shard_map import shard_map
        from concourse import bass2jax, mybir as mb

        bass2jax.install_neuronx_cc_hook()
        nc = _get_built(group_size, iters)
        self.nc = nc
        partition_name = (
            nc.partition_id_tensor.name if nc.partition_id_tensor else None
        )
        in_names, out_names, out_avals, zero_outs = [], [], [], []
        for alloc in nc.m.functions[0].allocations:
            if not isinstance(alloc, mb.MemoryLocationSet):
                continue
            name = alloc.memorylocations[0].name
            if alloc.kind == "ExternalInput":
                if name != partition_name:
                    in_names.append(name)
            elif alloc.kind == "ExternalOutput":
                out_names.append(name)
                shape = tuple(alloc.tensor_shape)
                dtype = mb.dt.np(alloc.dtype)
                out_avals.append(jax.core.ShapedArray(shape, dtype))
                zero_outs.append(np.zeros(shape, dtype))
        self.n_params = len(in_names)
        self.param_names = list(in_names)
        self.out_names = out_names
        self.out_avals = out_avals
        self.zero_outs = zero_outs
        all_in_names = in_names + out_names
        if partition_name is not None:
            all_in_names.append(partition_name)

        n_outs = len(out_names)
        donate = tuple(range(self.n_params, self.n_params + n_outs))

        def _body(*args):
            operands = list(args)
            if partition_name is not None:
                operands.append(bass2jax.partition_id_tensor())
            outs = bass2jax._bass_exec_p.bind(
                *operands,
                out_avals=tuple(out_avals),
                in_names=tuple(all_in_names),
                out_names=tuple(out_names),
                lowering_input_output_aliases=(),
                sim_require_finite=True,
                sim_require_nnan=True,
                nc=nc,
            )
            return tuple(outs)

        devices = jax.devices()[: self.N]
        self.mesh = Mesh(np.asarray(devices), ("core",))
        self.spec = PartitionSpec("core")
        self.sharding = NamedSharding(self.mesh, self.spec)
        in_specs = (self.spec,) * (self.n_params + n_outs)
        out_specs = (self.spec,) * n_outs
        self.fn = jax.jit(
            shard_map(
                _body, mesh=self.mesh, in_specs=in_specs, out_specs=out_specs,
                check_rep=False,
            ),
            donate_argnums=donate,
            keep_unused=True,
        )
        self._jax = jax

    def prepare(self, in_maps):
        """Concat per-core inputs and put on device; returns device arg list."""
        concat = [
            np.concatenate([np.asarray(m[n]) for m in in_maps], axis=0)
            for n in self.param_names
        ]
        return [self._jax.device_put(a, self.sharding) for a in concat]

    def _zeros(self):
        return [
            np.zeros((self.N * z.shape[0], *z.shape[1:]), z.dtype)
            for z in self.zero_outs
        ]

    def run_device(self, dev_args):
        outs = self.fn(*dev_args, *self._zeros())
        return [o.block_until_ready() for o in outs]

    def __call__(self, in_maps):
        outs = self.run_device(self.prepare(in_maps))
        res = []
        for c in range(self.N):
            res.append(
                {
                    n: np.asarray(outs[i]).reshape(
                        self.N, *self.out_avals[i].shape
                    )[c]
                    for i, n in enumerate(self.out_names)
                }
            )
        return res


_RUNNER = {}


def get_runner(group_size=2, iters=1):
    key = (group_size, iters)
    if key not in _RUNNER:
        _RUNNER[key] = _Runner(group_size, iters)
    return _RUNNER[key]


def make_in_maps(inputs, n_cores=8):
    """Host-side sharding + bf16 weight cast. Returns per-core input dicts."""
    bf16 = ml_dtypes.bfloat16
    x = np.asarray(inputs["x"], np.float32)
    shared = {
        "Wp": np.asarray(inputs["Wp"], np.float32).astype(bf16),
        "bp": np.asarray(inputs["bp"], np.float32),
        "Wq": np.asarray(inputs["Wq"], np.float32).astype(bf16),
        "Wk": np.asarray(inputs["Wk"], np.float32).astype(bf16),
        "Wv": np.asarray(inputs["Wv"], np.float32).astype(bf16),
        "Wo": np.asarray(inputs["Wo"], np.float32).astype(bf16),
        "bq": np.asarray(inputs["bq"], np.float32),
        "bk": np.asarray(inputs["bk"], np.float32),
        "bv": np.asarray(inputs["bv"], np.float32),
        "bo": np.asarray(inputs["bo"], np.float32),
        "ln_g": np.asarray(inputs["ln_g"], np.float32),
        "ln_b": np.asarray(inputs["ln_b"], np.float32),
        "W1": np.asarray(inputs["W1"], np.float32).astype(bf16),
        "b1": np.asarray(inputs["b1"], np.float32),
        "W2": np.asarray(inputs["W2"], np.float32).astype(bf16),
        "b2": np.asarray(inputs["b2"], np.float32),
    }
    in_maps = []
    for c in range(n_cores):
        b, half = c // 2, c % 2
        xT = np.ascontiguousarray(
            x[b, half * TOK : (half + 1) * TOK, :].T
        ).astype(bf16)
        in_maps.append({"xT": xT, **shared})
    return in_maps


def kernel(**inputs):
    runner = get_runner()
    results = runner(make_in_maps(inputs))
    ps = np.stack([results[c]["pooled"] for c in range(8)])  # [8, D]
    pooled = ps.reshape(4, 2, D).sum(axis=1) / 2048.0
    out = pooled @ np.asarray(inputs["Wout"], np.float32) + np.asarray(
        inputs["bout"], np.float32
    )
    return out.astype(np.float32)



# revision 39
# speedup vs baseline: 1.0408x; 1.0408x over previous
"""Trainium2 Bass kernel for nn_AttentionModel (4-layer transformer encoder).

Sharding: 8 cores = 4 batches x 2 sequence halves (1024 tokens/core).
Per layer each core AllGathers the feature-major hidden state within its
batch pair to get full-sequence k/v; attention + FFN are otherwise local.
"""
import sys

sys.path.insert(0, "/opt/trn_rl_repo")

import numpy as np
import ml_dtypes

import concourse.bass as bass
import concourse.tile as tile
from concourse import mybir, bacc
from concourse.masks import make_identity

DT = mybir.dt
AF = mybir.ActivationFunctionType
OP = mybir.AluOpType

P = 128
L, D, H, DFF, DIN, DOUT = 4, 512, 8, 2048, 64, 10
DK = D // H          # 64
NP = H // 2          # head pairs = 4
DS = D // P          # d slabs = 4
FS = DFF // P        # dff slabs = 16
TOK = 1024           # own tokens per core
NT = TOK // P        # 8 own token tiles
NG = 2               # token groups (512 each) per core
EPS = 1e-5
RSQRT_MAGIC = 0x5F3759DF

# Schraudolph fast-exp on DVE for a subset of softmax tiles: ACT's exp
# stream is the attention-phase bottleneck (1 elem/cycle/lane) while DVE
# has slack there. i = x*(0.125*log2e*2^23) + (127*2^23 - 486408);
# bitcast(i) ~ exp(0.125*x) with ~3% sawtooth rel err that averages out
# in the softmax weighted sum. Scores are |x| < ~100 so the exponent
# field can't under/overflow.
FEXP_C1 = 0.125 * 1.4426950408889634 * 8388608.0
FEXP_C2 = float(127 * 8388608 - 486408)
# (q, tt) softmax tiles routed to DVE. Measured on HW (iters-slope A/B):
# offloading 32 tiles/pass costs +7% (1.271 -> 1.364 ms/pass) — attention
# is not ACT-bound enough, and the DVE-queue insertion delays the
# normalize/LN ops PE waits on. TimelineSim agreed. Kept empty; the code
# path remains for future re-tuning (accuracy was fine: rel err 2.65e-3).
FEXP_QTT = frozenset()


def _bcast_ap(ap_1d, parts=P):
    """Broadcast a 1-D DRAM AP across partitions (DMA-side replication)."""
    return bass.AP(
        tensor=ap_1d.tensor, offset=ap_1d.offset, ap=[[0, parts], *ap_1d.ap]
    )


def _rsqrt_newton(nc, pool, var_ap, out_ap, w=1):
    """out = 1/sqrt(var + EPS) on DVE only (no ACT table switch). [P,w] f32."""
    x = pool.tile([P, w], DT.float32, tag=f"rs_x{w}")
    nc.vector.tensor_scalar_add(x[:], var_ap, EPS)
    t_i = pool.tile([P, w], DT.int32, tag=f"rs_i{w}")
    nc.vector.tensor_scalar(
        out=t_i[:], in0=x[:].bitcast(DT.int32), scalar1=1, scalar2=None,
        op0=OP.logical_shift_right,
    )
    nc.vector.tensor_scalar(
        out=out_ap.bitcast(DT.int32), in0=t_i[:], scalar1=-1, scalar2=RSQRT_MAGIC,
        op0=OP.mult, op1=OP.add,
    )
    tmp = pool.tile([P, w], DT.float32, tag=f"rs_t{w}")
    for _ in range(2):
        nc.vector.tensor_mul(tmp[:], out_ap, out_ap)
        nc.vector.tensor_mul(tmp[:], tmp[:], x[:])
        nc.vector.tensor_scalar(
            out=tmp[:], in0=tmp[:], scalar1=-0.5, scalar2=1.5, op0=OP.mult, op1=OP.add
        )
        nc.vector.tensor_mul(out_ap, out_ap, tmp[:])


def _ln_stats(nc, st, xs, tagsfx=""):
    """bn_stats/aggr for a group of [P, D] tiles -> packed [P, n, 2] mean/var."""
    n = len(xs)
    mvg = st.tile([P, n, 2], DT.float32, tag=f"ln_mvg{tagsfx}", name=f"mvg{tagsfx}")
    for i, x_ap in enumerate(xs):
        stats = st.tile([P, 6], DT.float32, tag="ln_st")
        nc.vector.bn_stats(stats[:], x_ap)
        nc.vector.bn_aggr(mvg[:, i, :], stats[:])
    return mvg


def _ln_finish(nc, st, mvg, xs, g_bc, b_bc, outs, tagsfx=""):
    """Vectorized Newton rsqrt + normalize/affine for the group."""
    n = len(xs)
    rsg = st.tile([P, n], DT.float32, tag=f"ln_rsg{tagsfx}", name=f"rsg{tagsfx}")
    _rsqrt_newton(nc, st, mvg[:, :, 1], rsg[:], w=n)
    for i, (x_ap, out_ap) in enumerate(zip(xs, outs)):
        nc.vector.tensor_scalar(
            out=x_ap, in0=x_ap, scalar1=mvg[:, i, 0:1], scalar2=rsg[:, i : i + 1],
            op0=OP.subtract, op1=OP.mult,
        )
        nc.vector.tensor_mul(x_ap, x_ap, g_bc)
        nc.vector.tensor_add(out_ap, x_ap, b_bc)


def _layernorm_group(nc, st, xs, g_bc, b_bc, outs, tagsfx=""):
    mvg = _ln_stats(nc, st, xs, tagsfx)
    _ln_finish(nc, st, mvg, xs, g_bc, b_bc, outs, tagsfx)


def _layernorm(nc, st, x_ap, g_bc, b_bc, out_ap):
    """LN over free dim D; x_ap [P, D] f32 -> out_ap (f32)."""
    stats = st.tile([P, 6], DT.float32, tag="ln_st")
    nc.vector.bn_stats(stats[:], x_ap)
    mv = st.tile([P, 2], DT.float32, tag="ln_mv")
    nc.vector.bn_aggr(mv[:], stats[:])
    rstd = st.tile([P, 1], DT.float32, tag="ln_rs")
    _rsqrt_newton(nc, st, mv[:, 1:2], rstd[:])
    nc.vector.tensor_scalar(
        out=x_ap, in0=x_ap, scalar1=mv[:, 0:1], scalar2=rstd[:],
        op0=OP.subtract, op1=OP.mult,
    )
    nc.vector.tensor_mul(x_ap, x_ap, g_bc)
    nc.vector.tensor_add(out_ap, x_ap, b_bc)


def build(group_size=2, fake_cc=False, iters=1):
    """Build the SPMD program. group_size=2 -> pairwise AG (real); 1 -> no-op
    AG (single-core numerics testing, S=1024). fake_cc replaces collectives
    with local DMA copies (perf-model variant for TimelineSim). iters>1
    repeats the full forward pass back-to-back inside the NEFF (identical
    output; used to measure steady-state per-pass device time by slope)."""
    GS = group_size
    S = GS * TOK                 # attended sequence length
    NC_TK = S // P               # tk tiles (16 or 8)
    NCH = S // 512               # 512-token key chunks (4 or 2)
    if GS == 2:
        groups = [[0, 1], [2, 3], [4, 5], [6, 7]]
    else:
        groups = [[c] for c in range(8)]

    nc = bacc.Bacc("TRN2", target_bir_lowering=False, debug=False)

    # ---- DRAM I/O ----
    xT_d = nc.dram_tensor("xT", [DIN, TOK], DT.bfloat16, kind="ExternalInput")
    Wp_d = nc.dram_tensor("Wp", [DIN, D], DT.bfloat16, kind="ExternalInput")
    bp_d = nc.dram_tensor("bp", [D], DT.float32, kind="ExternalInput")
    Wq_d = nc.dram_tensor("Wq", [L, D, D], DT.bfloat16, kind="ExternalInput")
    Wk_d = nc.dram_tensor("Wk", [L, D, D], DT.bfloat16, kind="ExternalInput")
    Wv_d = nc.dram_tensor("Wv", [L, D, D], DT.bfloat16, kind="ExternalInput")
    Wo_d = nc.dram_tensor("Wo", [L, D, D], DT.bfloat16, kind="ExternalInput")
    bq_d = nc.dram_tensor("bq", [L, D], DT.float32, kind="ExternalInput")
    bk_d = nc.dram_tensor("bk", [L, D], DT.float32, kind="ExternalInput")
    bv_d = nc.dram_tensor("bv", [L, D], DT.float32, kind="ExternalInput")
    bo_d = nc.dram_tensor("bo", [L, D], DT.float32, kind="ExternalInput")
    lg_d = nc.dram_tensor("ln_g", [L, D], DT.float32, kind="ExternalInput")
    lb_d = nc.dram_tensor("ln_b", [L, D], DT.float32, kind="ExternalInput")
    W1_d = nc.dram_tensor("W1", [L, D, DFF], DT.bfloat16, kind="ExternalInput")
    b1_d = nc.dram_tensor("b1", [L, DFF], DT.float32, kind="ExternalInput")
    W2_d = nc.dram_tensor("W2", [L, DFF, D], DT.bfloat16, kind="ExternalInput")
    b2_d = nc.dram_tensor("b2", [L, D], DT.float32, kind="ExternalInput")
    out_d = nc.dram_tensor("pooled", [D], DT.float32, kind="ExternalOutput")

    # AG bounce buffers (per layer)
    ag_in = [
        nc.dram_tensor(f"ag_in{l}", [NG, DS, P, 512], DT.bfloat16) for l in range(L)
    ]
    ag_out = [
        nc.dram_tensor(f"ag_out{l}", [NG, GS, DS, P, 512], DT.bfloat16)
        for l in range(L)
    ]

    from contextlib import ExitStack

    with tile.TileContext(nc) as tc:
        with ExitStack() as _es:
            sing = _es.enter_context(tc.tile_pool(name="sing", bufs=1))
            hp = _es.enter_context(tc.tile_pool(name="hp", bufs=2))
            hTp = _es.enter_context(tc.tile_pool(name="hTp", bufs=1))
            kvp = _es.enter_context(tc.tile_pool(name="kvp", bufs=1))
            chk = _es.enter_context(tc.tile_pool(name="chk", bufs=3))
            wts = _es.enter_context(tc.tile_pool(name="wts", bufs=1))
            biasp = _es.enter_context(tc.tile_pool(name="bias", bufs=1))
            lnp = _es.enter_context(tc.tile_pool(name="lnp", bufs=2))
            expp = _es.enter_context(tc.tile_pool(name="expp", bufs=4))
            rtp = _es.enter_context(tc.tile_pool(name="rtp", bufs=4))
            rcp = _es.enter_context(tc.tile_pool(name="rcp", bufs=2))
            st = _es.enter_context(tc.tile_pool(name="st", bufs=4))
            xw = _es.enter_context(tc.tile_pool(name="xw", bufs=4))
            # ---- singles ----
            ident = sing.tile([P, P], DT.float32)
            make_identity(nc, ident[:])
            ones_col = sing.tile([P, 1], DT.float32)
            nc.vector.memset(ones_col[:], 1.0)
            bp_bc = sing.tile([P, D], DT.float32)
            nc.sync.dma_start(bp_bc[:], _bcast_ap(bp_d[:]))
            xT_sb = sing.tile([DIN, TOK], DT.bfloat16)
            nc.sync.dma_start(xT_sb[:], xT_d[:])
            Wp_sb = sing.tile([DIN, D], DT.bfloat16)
            nc.sync.dma_start(Wp_sb[:], Wp_d[:])
            warm = sing.tile([1, 2], DT.float32)
            nc.vector.memset(warm[:], 0.0)
            nc.scalar.activation(warm[:, 1:2], warm[:, 0:1], AF.Exp)
            nc.vector.tensor_copy(warm[:, 0:1], warm[:, 1:2])

            # persistent activation tiles
            hT_sb = hTp.tile([P, DS, TOK], DT.bfloat16, tag="hT")
            hpT_sb = hTp.tile([P, DS, TOK], DT.bfloat16, tag="hpT")
            qT_sb = kvp.tile([P, NP, TOK], DT.bfloat16, tag="qT")
            kT_sb = kvp.tile([P, NP, S], DT.bfloat16, tag="kT")
            v_sb = kvp.tile([P, NC_TK, H, DK + 1], DT.bfloat16, tag="v")
            ctxT_sb = kvp.tile([P, NP, TOK], DT.bfloat16, tag="ctxT")
            # softmax-denominator ones column: written once — the per-layer
            # v writes only touch columns 0:DK, so it survives all layers
            # and iterations (was previously re-memset per layer, adding a
            # Pool launch + cross-engine sync into each layer's v path)
            nc.gpsimd.memset(v_sb[:, :, :, DK : DK + 1], 1.0)

            def transpose_to(dst_sb, src_ap, tt, ps_pool, tag="ps_t"):
                """PE-transpose a [P, D] f32 tile into dst_sb[:, :, tt*128...]"""
                ps_t = ps_pool.tile([P, D], DT.float32, tag=tag,
                                    name=f"tr_{dst_sb.tensor.name}_{tt}")
                for s in range(DS):
                    nc.tensor.transpose(
                        ps_t[:, s * P : (s + 1) * P],
                        src_ap[:, s * P : (s + 1) * P],
                        ident[:],
                    )
                nc.vector.tensor_copy(
                    dst_sb[:, :, tt * P : (tt + 1) * P].rearrange("p s t -> p s t"),
                    ps_t[:].rearrange("p (s t) -> p s t", s=DS),
                )

            def start_ag(l, g):
                nc.sync.dma_start(
                    ag_in[l][g].rearrange("s p t -> p s t"),
                    hT_sb[:, :, g * 512 : (g + 1) * 512],
                )
                if fake_cc:
                    for r in range(GS):
                        nc.sync.dma_start(ag_out[l][g][r], ag_in[l][g])
                else:
                    nc.gpsimd.collective_compute(
                        "AllGather",
                        OP.bypass,
                        replica_groups=groups,
                        ins=[ag_in[l][g].opt()],
                        outs=[ag_out[l][g].opt()],
                    )

            for _it in range(iters):
              # ---- input projection -> h(0), hT, AG(0) ----
              h_cur = hp.tile([P, NT, D], DT.float32, tag="h", name=f"h_cur{_it}")
              with tc.tile_pool(name=f"ps_x{_it}", bufs=2, space="PSUM") as ps_x:
                for g in range(NG):
                    for j in range(4):
                        tt = 4 * g + j
                        ps = ps_x.tile([P, D], DT.float32, tag="ps_h0")
                        nc.tensor.matmul(
                            ps[:], xT_sb[:, tt * P : (tt + 1) * P], Wp_sb[:],
                            start=True, stop=True,
                        )
                        nc.vector.tensor_add(h_cur[:, tt, :], ps[:], bp_bc[:])
                        transpose_to(hT_sb, h_cur[:, tt, :], tt, ps_x)
                    start_ag(0, g)

              # ---- layers ----
              for l in range(L):
                # -- weights / params for this layer --
                Wq_sb = wts.tile([P, DS, D], DT.bfloat16, tag="Wq")
                Wk_sb = wts.tile([P, DS, D], DT.bfloat16, tag="Wk")
                Wv_sb = wts.tile([P, DS, D], DT.bfloat16, tag="Wv")
                Wo_sb = wts.tile([P, DS, D], DT.bfloat16, tag="Wo")
                for w_sb, w_d in ((Wq_sb, Wq_d), (Wk_sb, Wk_d), (Wv_sb, Wv_d),
                                  (Wo_sb, Wo_d)):
                    nc.sync.dma_start(
                        w_sb[:], w_d[l].rearrange("(s p) e -> p s e", p=P)
                    )
                bq_c = biasp.tile([P, DS], DT.float32, tag="bq")
                nc.sync.dma_start(bq_c[:], bq_d[l].rearrange("(s p) -> p s", p=P))
                bk_c = biasp.tile([P, DS], DT.float32, tag="bk")
                nc.sync.dma_start(bk_c[:], bk_d[l].rearrange("(s p) -> p s", p=P))
                bv_bc = biasp.tile([P, D], DT.float32, tag="bv")
                nc.sync.dma_start(bv_bc[:], _bcast_ap(bv_d[l]))

                # -- qT (own tokens) --
                with tc.tile_pool(name=f"ps_q{_it}_{l}", bufs=2, space="PSUM") as ps_q:
                    for p_ in range(NP):
                        for q in range(2):
                            ps = ps_q.tile([P, 512], DT.float32, tag="ps_qT")
                            for s in range(DS):
                                nc.tensor.matmul(
                                    ps[:],
                                    Wq_sb[:, s, p_ * P : (p_ + 1) * P],
                                    hT_sb[:, s, q * 512 : (q + 1) * 512],
                                    start=(s == 0), stop=(s == DS - 1),
                                )
                            nc.vector.tensor_scalar_add(
                                qT_sb[:, p_, q * 512 : (q + 1) * 512],
                                ps[:], bq_c[:, p_ : p_ + 1],
                            )

                    # -- k/v from gathered full-sequence hT --
                    # key-column order = arrival order: group-0 chunks (whose
                    # AllGather completes earliest) take the low kT columns, so
                    # attention's ctx accumulation (tt ascending) never stalls
                    # on the late group-1 gather. Key permutation is safe:
                    # softmax is order-invariant and v uses the same order.
                    chunks = sorted(
                        [(ch, g) for ch in range(GS) for g in range(NG)],
                        key=lambda t: (t[1], t[0]),
                    )
                    for c, (ch, g) in enumerate(chunks):
                        hTf = chk.tile([P, DS, 512], DT.bfloat16, tag="hTf")
                        nc.sync.dma_start(
                            hTf[:], ag_out[l][g][ch].rearrange("s p t -> p s t")
                        )
                        for p_ in range(NP):
                            ps = ps_q.tile([P, 512], DT.float32, tag="ps_qT")
                            for s in range(DS):
                                nc.tensor.matmul(
                                    ps[:],
                                    Wk_sb[:, s, p_ * P : (p_ + 1) * P],
                                    hTf[:, s, :],
                                    start=(s == 0), stop=(s == DS - 1),
                                )
                            nc.vector.tensor_scalar_add(
                                kT_sb[:, p_, c * 512 : (c + 1) * 512],
                                ps[:], bk_c[:, p_ : p_ + 1],
                            )
                        for j in range(4):
                            tt = 4 * c + j
                            ps = ps_q.tile([P, 512], DT.float32, tag="ps_qT")
                            for s in range(DS):
                                nc.tensor.matmul(
                                    ps[:],
                                    hTf[:, s, j * P : (j + 1) * P],
                                    Wv_sb[:, s, :],
                                    start=(s == 0), stop=(s == DS - 1),
                                )
                            nc.vector.tensor_add(
                                v_sb[:, tt, :, 0:DK].rearrange("p h d -> p h d"),
                                ps[:].rearrange("p (h d) -> p h d", h=H),
                                bv_bc[:].rearrange("p (h d) -> p h d", h=H),
                            )

                # -- deferred big-weight loads: issued after the latency-
                # critical AG/kv-chunk DMAs so they don't contend for queues;
                # they complete during the long attention phase --
                W1_sb = wts.tile([P, DS, DFF], DT.bfloat16, tag="W1")
                nc.sync.dma_start(W1_sb[:], W1_d[l].rearrange("(s p) e -> p s e", p=P))
                W2_sb = wts.tile([P, FS, D], DT.bfloat16, tag="W2")
                nc.sync.dma_start(W2_sb[:], W2_d[l].rearrange("(s p) e -> p s e", p=P))
                b1_c = biasp.tile([P, FS], DT.float32, tag="b1")
                nc.sync.dma_start(b1_c[:], b1_d[l].rearrange("(s p) -> p s", p=P))
                bo_bc = biasp.tile([P, D], DT.float32, tag="bo")
                nc.sync.dma_start(bo_bc[:], _bcast_ap(bo_d[l]))
                b2_bc = biasp.tile([P, D], DT.float32, tag="b2")
                nc.sync.dma_start(b2_bc[:], _bcast_ap(b2_d[l]))
                lg_bc = lnp.tile([P, D], DT.float32, tag="lg")
                nc.sync.dma_start(lg_bc[:], _bcast_ap(lg_d[l]))
                lb_bc = lnp.tile([P, D], DT.float32, tag="lb")
                nc.sync.dma_start(lb_bc[:], _bcast_ap(lb_d[l]))

                # -- attention (q-outer; head pairs row-packed on PE) --
                # attn_out/LN1/h'-transposes for token group q are emitted
                # right after q's heads so they overlap the other q's
                # ACT-bound softmax work.
                h_ln1 = hp.tile([P, NT, D], DT.float32, tag="h")
                xs_g0, mvg_g0 = [], []
                with (
                    tc.tile_pool(name=f"ps_s{_it}_{l}", bufs=2, space="PSUM") as ps_s,
                    tc.tile_pool(name=f"ps_cx{_it}_{l}", bufs=2, space="PSUM") as ps_cx,
                ):
                    for q in range(2):
                        for p_ in range(NP):
                            if q == 1 and p_ == 1:
                                # group-0 attn_out + residual + stats ride the
                                # fast-release cx slots, overlapping q1's
                                # ACT-bound softmax (transposes/Newton stay in
                                # the FFN block: they are LN-gated)
                                for j4 in range(4):
                                    aops = ps_cx.tile(
                                        [P, D], DT.float32,
                                        tag=("ps_cx0", "ps_cx1")[j4 % 2],
                                        name=f"ao_{l}_{j4}",
                                    )
                                    for p2 in range(NP):
                                        nc.tensor.matmul(
                                            aops[:],
                                            ctxT_sb[:, p2, j4 * P : (j4 + 1) * P],
                                            Wo_sb[:, p2, :],
                                            start=(p2 == 0), stop=(p2 == NP - 1),
                                        )
                                    x = xw.tile([P, D], DT.float32, tag="x1",
                                                name=f"x_{l}_{j4}")
                                    nc.vector.tensor_add(
                                        x[:], aops[:], h_cur[:, j4, :]
                                    )
                                    nc.vector.tensor_add(x[:], x[:], bo_bc[:])
                                    xs_g0.append(x)
                                mvg_g0.append(
                                    _ln_stats(nc, st,
                                              [x[:] for x in xs_g0], tagsfx="a")
                                )
                                _ln_finish(
                                    nc, st, mvg_g0[0], [x[:] for x in xs_g0],
                                    lg_bc[:], lb_bc[:],
                                    [h_ln1[:, j4, :] for j4 in range(4)],
                                    tagsfx="a",
                                )
                            if q == 1 and p_ == NP - 1:
                                # group-0 h' transposes: inputs ready (LN-g0
                                # finished mid-q1), fast release; priority just
                                # below pairs 0-2 so the exp stream never
                                # starves. Frees FFN1(g0) to start at FFN-open.
                                for j4 in range(4):
                                    trp = ps_cx.tile(
                                        [P, D], DT.float32,
                                        tag=("ps_cx0", "ps_cx1")[j4 % 2],
                                        name=f"trg0_{l}_{j4}",
                                    )
                                    for s in range(DS):
                                        nc.tensor.transpose(
                                            trp[:, s * P : (s + 1) * P],
                                            h_ln1[:, j4, s * P : (s + 1) * P],
                                            ident[:],
                                        )
                                    nc.vector.tensor_copy(
                                        hpT_sb[:, :, j4 * P : (j4 + 1) * P],
                                        trp[:].rearrange("p (s t) -> p s t", s=DS),
                                    )
                            ctx0 = ps_cx.tile([P, 512], DT.float32, tag="ps_cx0",
                                              name=f"cx0_{l}_{q}_{p_}")
                            ctx1 = ps_cx.tile([P, 512], DT.float32, tag="ps_cx1",
                                              name=f"cx1_{l}_{q}_{p_}")
                            ctxs = (ctx0, ctx1)
                            for tt in range(NC_TK):
                                sc_ps = ps_s.tile([P, 1024], DT.float32, tag="ps_sc",
                                                  name=f"sc_{l}_{q}_{p_}_{tt}")
                                for sub in range(2):
                                    nc.tensor.matmul(
                                        sc_ps[:, sub * 512 : (sub + 1) * 512],
                                        kT_sb[
                                            sub * DK : (sub + 1) * DK,
                                            p_, tt * P : (tt + 1) * P,
                                        ],
                                        qT_sb[
                                            sub * DK : (sub + 1) * DK,
                                            p_, q * 512 : (q + 1) * 512,
                                        ],
                                        start=True, stop=True,
                                        tile_position=(sub * DK, 0),
                                    )
                                eT = expp.tile([P, 1024], DT.bfloat16, tag="expT")
                                if (q, tt) in FEXP_QTT:
                                    # bufs=1: producer+consumer both on DVE
                                    # in-order, so no WAR stall
                                    ei = expp.tile([P, 1024], DT.int32,
                                                   tag="expI", bufs=1)
                                    nc.vector.tensor_scalar(
                                        out=ei[:], in0=sc_ps[:],
                                        scalar1=FEXP_C1, scalar2=FEXP_C2,
                                        op0=OP.mult, op1=OP.add,
                                    )
                                    nc.vector.tensor_copy(
                                        eT[:], ei[:].bitcast(DT.float32)
                                    )
                                else:
                                    nc.scalar.activation(eT[:], sc_ps[:],
                                                         AF.Exp, scale=0.125)
                                for sub in range(2):
                                    nc.tensor.matmul(
                                        ctxs[sub][0 : DK + 1, :],
                                        v_sb[:, tt, 2 * p_ + sub, :],
                                        eT[:, sub * 512 : (sub + 1) * 512],
                                        start=(tt == 0), stop=(tt == NC_TK - 1),
                                    )
                            for sub in range(2):
                                recip = rcp.tile([1, 512], DT.float32, tag="recip")
                                nc.vector.reciprocal(
                                    recip[:], ctxs[sub][DK : DK + 1, :]
                                )
                                rb = rcp.tile([DK, 512], DT.float32, tag="rb")
                                nc.gpsimd.partition_broadcast(rb[:], recip[:])
                                nc.vector.tensor_mul(
                                    ctxT_sb[
                                        sub * DK : (sub + 1) * DK,
                                        p_, q * 512 : (q + 1) * 512,
                                    ],
                                    ctxs[sub][0:DK, :], rb[:],
                                )

                # -- FFN + LN2 (+ next layer's hT/AG) --
                h_nxt = hp.tile([P, NT, D], DT.float32, tag="h")
                with (
                    tc.tile_pool(name=f"ps_d{_it}_{l}", bufs=2, space="PSUM") as ps_d,
                    tc.tile_pool(name=f"ps_r{_it}_{l}", bufs=2, space="PSUM") as ps_r,
                    tc.tile_pool(name=f"ps_f{_it}_{l}", bufs=1, space="PSUM") as ps_f,
                ):
                    for g4 in range(2):
                        if g4 == 0:
                            pass  # LN finished inside the attention window
                        else:
                            xs = []
                            for j4 in range(4):
                                tt = 4 * g4 + j4
                                ps = ps_d.tile([P, D], DT.float32, tag="ps_aot",
                                               name=f"ao_{l}_{tt}")
                                for p_ in range(NP):
                                    nc.tensor.matmul(
                                        ps[:],
                                        ctxT_sb[:, p_, tt * P : (tt + 1) * P],
                                        Wo_sb[:, p_, :],
                                        start=(p_ == 0), stop=(p_ == NP - 1),
                                    )
                                x = xw.tile([P, D], DT.float32, tag="x1",
                                            name=f"x_{l}_{tt}")
                                nc.vector.tensor_add(x[:], ps[:], h_cur[:, tt, :])
                                nc.vector.tensor_add(x[:], x[:], bo_bc[:])
                                xs.append(x)
                            _layernorm_group(
                                nc, st, [x[:] for x in xs], lg_bc[:], lb_bc[:],
                                [h_ln1[:, 4 * g4 + j4, :] for j4 in range(4)],
                                tagsfx="a",
                            )
                        if g4 == 1:
                            for j4 in range(4):
                                tt = 4 * g4 + j4
                                transpose_to(hpT_sb, h_ln1[:, tt, :], tt, ps_d,
                                             "ps_aot")
                    for g in range(NG):
                        outs = []
                        for j in range(4):
                            ot = ps_f.tile(
                                [P, D], DT.float32, tag=f"ps_ffn{j}",
                                name=f"ps_ffn{l}_{g}_{j}",
                            )
                            outs.append(ot)
                        for f in range(FS):
                            ps = ps_r.tile([P, 512], DT.float32, tag="ps_r")
                            for s in range(DS):
                                nc.tensor.matmul(
                                    ps[:],
                                    W1_sb[:, s, f * P : (f + 1) * P],
                                    hpT_sb[:, s, g * 512 : (g + 1) * 512],
                                    start=(s == 0), stop=(s == DS - 1),
                                )
                            rT = rtp.tile([P, 512], DT.bfloat16, tag="rT")
                            nc.scalar.activation(
                                rT[:], ps[:], AF.Relu, bias=b1_c[:, f : f + 1]
                            )
                            for j in range(4):
                                nc.tensor.matmul(
                                    outs[j][:],
                                    rT[:, j * P : (j + 1) * P],
                                    W2_sb[:, f, :],
                                    start=(f == 0), stop=(f == FS - 1),
                                )
                        xs2 = []
                        for j in range(4):
                            tt = 4 * g + j
                            x = xw.tile([P, D], DT.float32, tag="x1",
                                        name=f"x2_{l}_{g}_{j}")
                            nc.vector.tensor_add(x[:], outs[j][:], h_ln1[:, tt, :])
                            nc.vector.tensor_add(x[:], x[:], b2_bc[:])
                            xs2.append(x)
                        _layernorm_group(
                            nc, st, [x[:] for x in xs2], lg_bc[:], lb_bc[:],
                            [h_nxt[:, 4 * g + j, :] for j in range(4)],
                            tagsfx="f",
                        )
                        if l < L - 1:
                            for j in range(4):
                                tt = 4 * g + j
                                transpose_to(hT_sb, h_nxt[:, tt, :], tt, ps_d,
                                             "ps_aot")
                        if l < L - 1:
                            start_ag(l + 1, g)
                h_cur = h_nxt

              # ---- pooled sum over own tokens ----
              with tc.tile_pool(name=f"ps_p{_it}", bufs=1, space="PSUM") as ps_p:
                ps = ps_p.tile([1, D], DT.float32, tag="ps_pool")
                for tt in range(NT):
                    nc.tensor.matmul(
                        ps[:], ones_col[:], h_cur[:, tt, :],
                        start=(tt == 0), stop=(tt == NT - 1),
                    )
                osb = sing.tile([1, D], DT.float32, tag="osb", name=f"osb{_it}")
                nc.vector.tensor_copy(osb[:], ps[:])
                nc.sync.dma_start(out_d[None, :], osb[:])

    nc.compile()
    return nc


_BUILT = {}


def _get_built(group_size=2, iters=1):
    key = (group_size, iters)
    if key not in _BUILT:
        _BUILT[key] = build(group_size, iters=iters)
    return _BUILT[key]


class _Runner:
    """Persistent jitted SPMD executor (jit + NEFF compiled once)."""

    N = 8

    def __init__(self, group_size=2, iters=1):
        import jax
        from jax.sharding import Mesh, PartitionSpec, NamedSharding
        from jax.experimental.shard_map import shard_map
        from concourse import bass2jax, mybir as mb

        bass2jax.install_neuronx_cc_hook()
        nc = _get_built(group_size, iters)
        self.nc = nc
        partition_name = (
            nc.partition_id_tensor.name if nc.partition_id_tensor else None
        )
        in_names, out_names, out_avals, zero_outs = [], [], [], []
        for alloc in nc.m.functions[0].allocations:
            if not isinstance(alloc, mb.MemoryLocationSet):
                continue
            name = alloc.memorylocations[0].name
            if alloc.kind == "ExternalInput":
                if name != partition_name:
                    in_names.append(name)
            elif alloc.kind == "ExternalOutput":
                out_names.append(name)
                shape = tuple(alloc.tensor_shape)
                dtype = mb.dt.np(alloc.dtype)
                out_avals.append(jax.core.ShapedArray(shape, dtype))
                zero_outs.append(np.zeros(shape, dtype))
        self.n_params = len(in_names)
        self.param_names = list(in_names)
        self.out_names = out_names
        self.out_avals = out_avals
        self.zero_outs = zero_outs
        all_in_names = in_names + out_names
        if partition_name is not None:
            all_in_names.append(partition_name)

        n_outs = len(out_names)
        donate = tuple(range(self.n_params, self.n_params + n_outs))

        def _body(*args):
            operands = list(args)
            if partition_name is not None:
                operands.append(bass2jax.partition_id_tensor())
            outs = bass2jax._bass_exec_p.bind(
                *operands,
                out_avals=tuple(out_avals),
                in_names=tuple(all_in_names),
                out_names=tuple(out_names),
                lowering_input_output_aliases=(),
                sim_require_finite=True,
                sim_require_nnan=True,
                nc=nc,
            )
            return tuple(outs)

        devices = jax.devices()[: self.N]
        self.mesh = Mesh(np.asarray(devices), ("core",))
        self.spec = PartitionSpec("core")
        self.sharding = NamedSharding(self.mesh, self.spec)
        in_specs = (self.spec,) * (self.n_params + n_outs)
        out_specs = (self.spec,) * n_outs
        self.fn = jax.jit(
            shard_map(
                _body, mesh=self.mesh, in_specs=in_specs, out_specs=out_specs,
                check_rep=False,
            ),
            donate_argnums=donate,
            keep_unused=True,
        )
        self._jax = jax

    def prepare(self, in_maps):
        """Concat per-core inputs and put on device; returns device arg list."""
        concat = [
            np.concatenate([np.asarray(m[n]) for m in in_maps], axis=0)
            for n in self.param_names
        ]
        return [self._jax.device_put(a, self.sharding) for a in concat]

    def _zeros(self):
        return [
            np.zeros((self.N * z.shape[0], *z.shape[1:]), z.dtype)
            for z in self.zero_outs
        ]

    def run_device(self, dev_args):
        outs = self.fn(*dev_args, *self._zeros())
        return [o.block_until_ready() for o in outs]

    def __call__(self, in_maps):
        outs = self.run_device(self.prepare(in_maps))
        res = []
        for c in range(self.N):
            res.append(
                {
                    n: np.asarray(outs[i]).reshape(
                        self.N, *self.out_avals[i].shape
                    )[c]
                    for i, n in enumerate(self.out_names)
                }
            )
        return res


_RUNNER = {}


def get_runner(group_size=2, iters=1):
    key = (group_size, iters)
    if key not in _RUNNER:
        _RUNNER[key] = _Runner(group_size, iters)
    return _RUNNER[key]


def make_in_maps(inputs, n_cores=8):
    """Host-side sharding + bf16 weight cast. Returns per-core input dicts."""
    bf16 = ml_dtypes.bfloat16
    x = np.asarray(inputs["x"], np.float32)
    shared = {
        "Wp": np.asarray(inputs["Wp"], np.float32).astype(bf16),
        "bp": np.asarray(inputs["bp"], np.float32),
        "Wq": np.asarray(inputs["Wq"], np.float32).astype(bf16),
        "Wk": np.asarray(inputs["Wk"], np.float32).astype(bf16),
        "Wv": np.asarray(inputs["Wv"], np.float32).astype(bf16),
        "Wo": np.asarray(inputs["Wo"], np.float32).astype(bf16),
        "bq": np.asarray(inputs["bq"], np.float32),
        "bk": np.asarray(inputs["bk"], np.float32),
        "bv": np.asarray(inputs["bv"], np.float32),
        "bo": np.asarray(inputs["bo"], np.float32),
        "ln_g": np.asarray(inputs["ln_g"], np.float32),
        "ln_b": np.asarray(inputs["ln_b"], np.float32),
        "W1": np.asarray(inputs["W1"], np.float32).astype(bf16),
        "b1": np.asarray(inputs["b1"], np.float32),
        "W2": np.asarray(inputs["W2"], np.float32).astype(bf16),
        "b2": np.asarray(inputs["b2"], np.float32),
    }
    in_maps = []
    for c in range(n_cores):
        b, half = c // 2, c % 2
        xT = np.ascontiguousarray(
            x[b, half * TOK : (half + 1) * TOK, :].T
        ).astype(bf16)
        in_maps.append({"xT": xT, **shared})
    return in_maps


def kernel(**inputs):
    runner = get_runner()
    results = runner(make_in_maps(inputs))
    ps = np.stack([results[c]["pooled"] for c in range(8)])  # [8, D]
    pooled = ps.reshape(4, 2, D).sum(axis=1) / 2048.0
    out = pooled @ np.asarray(inputs["Wout"], np.float32) + np.asarray(
        inputs["bout"], np.float32
    )
    return out.astype(np.float32)

